# revision 16
# baseline (speedup 1.0000x reference)
"""3x3 SAME conv (B=32, Cin=128, H=W=64, Cout=256) + bias + relu on 8 trn2 cores.

Strategy: data-parallel over batch (4 images per core, no collectives).
Per image, implicit GEMM: the input lives in SBUF as a zero-padded
[Cin=128, 66, 66] bf16 tile; for each of the 9 taps a [128cin x 128cout]
bf16 weight slice multiplies a shifted [128, 8rows*64cols] window,
accumulating fp32 in PSUM. bf16 operands enable the compiler's Fast
Weight Load path (4-XBUS LDWEIGHTS), so the per-matmul weight load hides
behind the previous matmul's 512-column stream and the PE runs at its
~213ns/matmul issue rate instead of ~242ns with f32r's 4-byte loads.
Bias+relu are fused on the scalar engine straight out of PSUM.

The host pre-pads the input and pre-casts input+weights to bf16 (RNE),
so the device does zero DVE work on the data path. Startup: the PE HAM
warmup runs on a memset tile (no DMA dependency), weights land per-tap
on the scalar HWDGE queue so the first row-group's tap chain can chase
them, image 0 lands in row bands on the sync HWDGE queue, and images
1-3 prefetch in the background on gpsimd's software DGE.
"""

from contextlib import ExitStack

import ml_dtypes
import numpy as np

import concourse.bass as bass
import concourse.tile as tile
from concourse import bacc, mybir
from concourse.bass_utils import run_bass_kernel_spmd

N_CORES = 8
B, C_IN, H, W = 32, 128, 64, 64
C_OUT, K = 256, 3
B_LOC = B // N_CORES          # images per core
N_CHUNK = C_OUT // 128        # cout chunks of 128
ROWS_PER_MM = 8               # 8 rows x 64 cols = 512 moving elements
N_RG = H // ROWS_PER_MM       # row groups per image
HP, WP = H + 2, W + 2         # padded

_COMPILED = None


def _build():
    nc = bacc.Bacc("TRN2", target_bir_lowering=False, debug=False,
                   num_devices=N_CORES)

    inp = nc.dram_tensor("inp", [B_LOC, C_IN, HP, WP], mybir.dt.bfloat16,
                         kind="ExternalInput").ap()
    wt = nc.dram_tensor("wt", [N_CHUNK, C_IN, K * K, 128], mybir.dt.bfloat16,
                        kind="ExternalInput").ap()
    bias2 = nc.dram_tensor("bias2", [128, N_CHUNK], mybir.dt.float32,
                           kind="ExternalInput").ap()
    out = nc.dram_tensor("out", [B_LOC, C_OUT, H, W], mybir.dt.float32,
                         kind="ExternalOutput").ap()

    with tile.TileContext(nc) as tc, ExitStack() as ctx:
        consts = ctx.enter_context(tc.tile_pool(name="consts", bufs=1))
        pads = ctx.enter_context(tc.tile_pool(name="pads", bufs=1))
        outs = ctx.enter_context(tc.tile_pool(name="outs", bufs=6))
        psums = ctx.enter_context(tc.tile_pool(name="psums", bufs=6,
                                               space="PSUM"))
        wps = ctx.enter_context(tc.tile_pool(name="wps", bufs=1,
                                             space="PSUM"))

        # Weights on the scalar HWDGE ring, chunk-major so each cout-chunk
        # is one contiguous 2304B-per-channel piece: chunk 0 (295KB) gates
        # the stream start ~1.2us earlier than the full 590KB would (the
        # early transfers run at the ~358GB/s HBM limit alongside image 0's
        # first band); chunk 1 lands before its first use at +1.9us.
        w_r = consts.tile([128, N_CHUNK, K * K, 128], mybir.dt.bfloat16,
                          tag="w_r")
        b_sb = consts.tile([128, N_CHUNK], mybir.dt.float32, tag="b_sb")
        nc.scalar.dma_start(out=w_r[:, 0], in_=wt[0])
        nc.scalar.dma_start(out=b_sb[:], in_=bias2[:])
        nc.scalar.dma_start(out=w_r[:, 1], in_=wt[1])

        # Bridge the ~1us between PE dispatch-ready (~8.3us) and data-ready
        # (~9.3us) with two matmuls on a memset tile. A longer dummy warmup
        # would push the real stream out instead of helping; the HAM clock
        # ramp rides the front of the real stream either way.
        warm = consts.tile([128, 512], mybir.dt.bfloat16, tag="warm")
        nc.vector.memset(warm[:], 0.0)
        wpsum = wps.tile([128, ROWS_PER_MM * W], mybir.dt.float32,
                         tag="wpsum")
        for i in range(6):
            nc.tensor.matmul(wpsum[:], warm[:, 0:128], warm[:],
                             start=True, stop=True)

        pimgs = [pads.tile([128, HP, WP], mybir.dt.bfloat16,
                           name=f"pimg{i}", tag=f"pimg{i}")
                 for i in range(B_LOC)]

        # Image 0 in row bands on the sync ring, sized so band s lands just
        # before the row groups that read it start (rowgroup r reads padded
        # rows 8r..8r+9).
        bounds = [0, 10, 18, 34, 50, HP]
        for s in range(len(bounds) - 1):
            nc.sync.dma_start(out=pimgs[0][:, bounds[s]:bounds[s + 1], :],
                              in_=inp[0, :, bounds[s]:bounds[s + 1], :])

        # Images 1-3 prefetch on gpsimd's SWDGE — but only after ~3.5us of
        # scratch memsets on the gpsimd sequencer. The 16 SDMA engines
        # round-robin between queues at packet granularity, so an immediate
        # 3.3MB prefetch would halve the bandwidth of the stream-start
        # pieces (weights + band 0) exactly when they gate the PE.
        scratch = consts.tile([128, 2048], mybir.dt.bfloat16, tag="scratch")
        nc.gpsimd.memset(scratch[:], 0.0)
        nc.gpsimd.memset(scratch[:], 1.0)
        nc.gpsimd.memset(scratch[:], 2.0)
        for b in range(1, B_LOC):
            nc.gpsimd.dma_start(out=pimgs[b][:], in_=inp[b])

        # Chunk-outer: all 8 rowgroups of chunk 0 (~15.5us of matmuls) run
        # before the first use of the chunk-1 weights, which land mid-sweep.
        for b in range(B_LOC):
            pimg = pimgs[b]
            for c in range(N_CHUNK):
                for r in range(N_RG):
                    acc = psums.tile([128, ROWS_PER_MM * W], mybir.dt.float32,
                                     tag="acc")
                    y0 = r * ROWS_PER_MM
                    for t in range(K * K):
                        kh, kw = divmod(t, K)
                        rhs = pimg[:, y0 + kh:y0 + kh + ROWS_PER_MM,
                                   kw:kw + W]
                        nc.tensor.matmul(acc[:],
                                         w_r[:, c, t, :],
                                         rhs,
                                         start=(t == 0), stop=(t == K * K - 1))
                    o = outs.tile([128, ROWS_PER_MM, W], mybir.dt.float32,
                                  tag="o")
                    nc.scalar.activation(o[:], acc[:].rearrange(
                        "p (h w) -> p h w", h=ROWS_PER_MM),
                        mybir.ActivationFunctionType.Relu,
                        bias=b_sb[:, c:c + 1], scale=1.0)
                    last = (b == B_LOC - 1 and r == N_RG - 1
                            and c == N_CHUNK - 1)
                    if last:
                        # Halve the exposed tail: split the final store
                        # across both HWDGE rings (sync is idle by now).
                        h2 = ROWS_PER_MM // 2
                        nc.sync.dma_start(
                            out=out[b, c * 128:(c + 1) * 128,
                                    y0:y0 + h2, :],
                            in_=o[:, 0:h2])
                        nc.scalar.dma_start(
                            out=out[b, c * 128:(c + 1) * 128,
                                    y0 + h2:y0 + ROWS_PER_MM, :],
                            in_=o[:, h2:ROWS_PER_MM])
                    else:
                        nc.scalar.dma_start(
                            out=out[b, c * 128:(c + 1) * 128,
                                    y0:y0 + ROWS_PER_MM, :],
                            in_=o[:])

    nc.compile()
    return nc


def _get_compiled():
    global _COMPILED
    if _COMPILED is None:
        _COMPILED = _build()
    return _COMPILED


def _run(inp, weight, bias, trace=False):
    inp = np.asarray(inp, dtype=np.float32)
    weight = np.asarray(weight, dtype=np.float32)
    bias = np.asarray(bias, dtype=np.float32)

    # Zero-pad to 66x66 and cast to bf16 host-side.
    inp_p = np.zeros((B, C_IN, HP, WP), dtype=np.float32)
    inp_p[:, :, 1:H + 1, 1:W + 1] = inp
    inp_p = inp_p.astype(ml_dtypes.bfloat16)

    # weight [C_OUT, C_IN*K*K] -> [N_CHUNK, C_IN, K*K, 128] (chunk-major
    # lhsT layout: wt[c, ci, t, m] = weight[c*128+m, ci*9+t])
    wt = np.ascontiguousarray(
        weight.reshape(N_CHUNK, 128, C_IN, K * K).transpose(0, 2, 3, 1)
    ).astype(ml_dtypes.bfloat16)
    # bias [C_OUT] -> [128, N_CHUNK]: bias2[p, c] = bias[c*128 + p]
    bias2 = np.ascontiguousarray(bias.reshape(N_CHUNK, 128).T)

    nc = _get_compiled()
    in_maps = [
        {"inp": inp_p[i * B_LOC:(i + 1) * B_LOC], "wt": wt, "bias2": bias2}
        for i in range(N_CORES)
    ]
    res = run_bass_kernel_spmd(nc, in_maps, list(range(N_CORES)), trace=trace)
    full = np.concatenate([res.results[i]["out"] for i in range(N_CORES)],
                          axis=0)
    return full, res


def kernel(inp, weight, bias):
    full, _ = _run(inp, weight, bias, trace=False)
    return full
